# revision 7
# baseline (speedup 1.0000x reference)
"""Distributed self-attention kernel for Trainium2, 8 NeuronCores.

Strategy (sequence-parallel, per the module's own sharding):
  - Shard S=3072 across 8 cores (384 rows each).
  - Each core computes its q/k/v projection chunk with bf16 matmuls
    (fp32 accumulation in PSUM). k is produced transposed (k^T: [dim, seq]),
    v in natural layout ([seq, dim]).
  - One AllGather shares every core's k^T and v chunks (bf16, 1.5 MB/rank).
  - Each core runs attention for its own 384 queries against all 3072 keys
    in transposed-score layout: s^T[key, query] = k^T.T @ q^T per head, so
    the P@V contraction needs no transpose of P. exp() on ScalarE (scale
    fused, no max-subtraction needed: |qk/8| < ~3 for any sane input since
    the softmax is over well-scaled dot products; exp is safely in range).
  - Softmax denominator comes free as a 65th output row of the P@V matmul
    (a ones-column is appended to V in SBUF).
  - Per-head context is transposed back with PE transpose-mode, normalized
    by the denominator on VectorE, and DMA'd out as this core's [384, 1024]
    slice of the output. Host concatenates the 8 slices.
"""

import numpy as np
import ml_dtypes

import concourse.bass as bass
import concourse.bacc as bacc
import concourse.mybir as mybir
import concourse.tile as tile
from concourse import bass_utils

F32 = mybir.dt.float32
BF16 = mybir.dt.bfloat16
AF = mybir.ActivationFunctionType

N_CORES = 8
B, S, HID = 1, 3072, 1024
NH, HD = 16, 64
SC = S // N_CORES          # 384 sequence rows per core
QT = SC // 128             # 3 query sub-tiles per core
KT = S // 128              # 24 key tiles globally
KIN = HID + 1              # augmented contraction (bias row)
NG = KT // 3               # 8 groups of 3 key tiles for batched exp

_KSZ = 8 * 128 * SC        # elements in k^T block of the AG payload
_VSZ = SC * HID            # elements in v block
_PAYLOAD = _KSZ + _VSZ     # 786432 elements (bf16) per rank

_cache: dict = {}


def _build(with_mask: bool):
    nc = bacc.Bacc("TRN2", target_bir_lowering=False, debug=False,
                   num_devices=N_CORES)

    xt = nc.dram_tensor("xt", [KIN, SC], BF16, kind="ExternalInput")
    w = nc.dram_tensor("w", [3, KIN, HID], BF16, kind="ExternalInput")
    ident = nc.dram_tensor("ident", [128, 128], F32, kind="ExternalInput")
    if with_mask:
        maskt = nc.dram_tensor("maskt", [128, KT], F32, kind="ExternalInput")
    out = nc.dram_tensor("out", [SC, HID], F32, kind="ExternalOutput")

    with tile.TileContext(nc) as tc:
        with (
            tc.tile_pool(name="persist", bufs=1) as pp,
            tc.tile_pool(name="dram", bufs=1, space="DRAM") as dram,
        ):
            kvin = dram.tile([_PAYLOAD], BF16)
            kvout = dram.tile([N_CORES, _PAYLOAD], BF16, addr_space="Shared")

            # ---- persistent SBUF tensors ----
            xsb = pp.tile([128, 9 * SC], BF16)       # x^T, 9 contraction slices
            qsb = [pp.tile([128, SC], BF16, name=f"qsb{m}") for m in range(8)]
            ksb = [pp.tile([128, S], BF16, name=f"ksb{h}") for h in range(8)]
            vsb = [pp.tile([128, 16 * 65], BF16, name=f"vsb{k}") for k in range(KT)]
            idsb = pp.tile([128, 128], F32)
            ctxsb = [pp.tile([65, SC], F32, name=f"ctxsb{h}") for h in range(NH)]
            osb = [pp.tile([128, HID], F32, name=f"osb{t}") for t in range(QT)]
            if with_mask:
                msb = pp.tile([128, KT], F32)
                nc.sync.dma_start(msb[:], maskt[:])

            nc.sync.dma_start(idsb[:], ident[:])

            # x^T load: 8 full slices + 1-row bias slice
            for j in range(8):
                nc.sync.dma_start(xsb[:, j * SC:(j + 1) * SC],
                                  xt[j * 128:(j + 1) * 128, :])
            nc.sync.dma_start(xsb[0:1, 8 * SC:9 * SC], xt[1024:1025, :])

            # ones columns of v tiles (col 64 of each 65-wide head block)
            for k in range(KT):
                nc.gpsimd.memset(
                    vsb[k].rearrange("p (h y) -> p h y", y=65)[:, :, 64:65], 1.0)

            # ---- phase A: projections ----
            with (
                tc.tile_pool(name="wpool", bufs=6) as wpool,
                tc.tile_pool(name="stg", bufs=4) as stg,
                tc.tile_pool(name="ppsum", bufs=4, space="PSUM") as ppsum,
            ):
                def load_w(proj, j):
                    if j < 8:
                        wt = wpool.tile([128, HID], BF16, tag="w", bufs=12,
                                        name=f"w{proj}_{j}")
                        nc.sync.dma_start(wt[:], w[proj, j * 128:(j + 1) * 128, :])
                    else:
                        wt = wpool.tile([1, HID], BF16, tag="wb", bufs=3,
                                        name=f"wb{proj}")
                        nc.sync.dma_start(wt[:], w[proj, HID:HID + 1, :])
                    return wt

                # k^T (proj 1) then v (proj 2) first, so the AllGather can
                # launch while q^T (proj 0) still computes.
                wk = [load_w(1, j) for j in range(9)]
                for m in range(8):
                    pk = ppsum.tile([128, SC], F32, tag="pk")
                    for j in range(9):
                        rows = 128 if j < 8 else 1
                        nc.tensor.matmul(
                            pk[:], wk[j][:rows, m * 128:(m + 1) * 128],
                            xsb[:rows, j * SC:(j + 1) * SC],
                            start=(j == 0), stop=(j == 8))
                    kst = stg.tile([128, SC], BF16, tag="kst")
                    nc.vector.tensor_copy(kst[:], pk[:])
                    nc.sync.dma_start(
                        kvin[m * 128 * SC:(m + 1) * 128 * SC]
                        .rearrange("(p x) -> p x", x=SC), kst[:])

                wv = [load_w(2, j) for j in range(9)]
                for st in range(QT):
                    for half in range(2):
                        pv = ppsum.tile([128, 512], F32, tag="pv")
                        for j in range(9):
                            rows = 128 if j < 8 else 1
                            nc.tensor.matmul(
                                pv[:],
                                xsb[:rows, j * SC + st * 128: j * SC + (st + 1) * 128],
                                wv[j][:rows, half * 512:(half + 1) * 512],
                                start=(j == 0), stop=(j == 8))
                        vst = stg.tile([128, 512], BF16, tag="vst")
                        nc.vector.tensor_copy(vst[:], pv[:])
                        vblk = (kvin[_KSZ + st * 128 * HID:
                                     _KSZ + (st + 1) * 128 * HID]
                                .rearrange("(p x) -> p x", x=HID))
                        nc.sync.dma_start(
                            vblk[:, half * 512:(half + 1) * 512], vst[:])

                nc.gpsimd.collective_compute(
                    "AllGather",
                    mybir.AluOpType.bypass,
                    replica_groups=[list(range(N_CORES))],
                    ins=[kvin[:].opt()],
                    outs=[kvout[:].opt()],
                )

                wq = [load_w(0, j) for j in range(9)]
                for m in range(8):
                    pq = ppsum.tile([128, SC], F32, tag="pk")
                    for j in range(9):
                        rows = 128 if j < 8 else 1
                        nc.tensor.matmul(
                            pq[:], wq[j][:rows, m * 128:(m + 1) * 128],
                            xsb[:rows, j * SC:(j + 1) * SC],
                            start=(j == 0), stop=(j == 8))
                    nc.vector.tensor_copy(qsb[m][:], pq[:])

            # ---- phase B: spread gathered K/V into SBUF ----
            for hp in range(8):
                for r in range(N_CORES):
                    nc.sync.dma_start(
                        ksb[hp][:, r * SC:(r + 1) * SC],
                        kvout[r, hp * 128 * SC:(hp + 1) * 128 * SC]
                        .rearrange("(p x) -> p x", x=SC))
            for k in range(KT):
                r, st = k // QT, k % QT
                src = (kvout[r, _KSZ + st * 128 * HID: _KSZ + (st + 1) * 128 * HID]
                       .rearrange("(p h y) -> p h y", p=128, y=HD))
                nc.sync.dma_start(
                    vsb[k].rearrange("p (h y) -> p h y", y=65)[:, :, 0:64], src)

            # ---- phase C: attention, head pairs ----
            with (
                tc.tile_pool(name="spool", bufs=2, space="PSUM") as spool,
                tc.tile_pool(name="cpool", bufs=2, space="PSUM") as cpool,
                tc.tile_pool(name="ppool", bufs=3) as ppool,
            ):
                for h in range(NH):
                    hp, e = h // 2, h % 2
                    ctx = cpool.tile([65, SC], F32, tag="ctx", name=f"ctx{h}")
                    for g in range(NG):
                        sp = spool.tile([128, 1536], F32, tag="sp",
                                        name=f"sp{h}_{g}")
                        for j in range(3):
                            kt = g * 3 + j
                            nc.tensor.matmul(
                                sp[:, j * 512: j * 512 + SC],
                                ksb[hp][e * 64:(e + 1) * 64,
                                        kt * 128:(kt + 1) * 128],
                                qsb[hp][e * 64:(e + 1) * 64, :],
                                start=True, stop=True)
                        pt = ppool.tile([128, 3 * SC], BF16, tag="pt",
                                        name=f"pt{h}_{g}")
                        src3 = sp.rearrange("p (g x) -> p g x", x=512)[:, :, 0:SC]
                        dst3 = pt.rearrange("p (g x) -> p g x", x=SC)
                        if with_mask:
                            for j in range(3):
                                kt = g * 3 + j
                                nc.scalar.activation(
                                    dst3[:, j, :], src3[:, j, :], AF.Exp,
                                    bias=msb[:, kt:kt + 1], scale=0.125)
                        else:
                            nc.scalar.activation(dst3, src3, AF.Exp, scale=0.125)
                        for j in range(3):
                            kt = g * 3 + j
                            nc.tensor.matmul(
                                ctx[:],
                                vsb[kt][:, h * 65:(h + 1) * 65],
                                pt[:, j * SC:(j + 1) * SC],
                                start=(g == 0 and j == 0),
                                stop=(g == NG - 1 and j == 2))
                    nc.vector.tensor_copy(ctxsb[h][:], ctx[:])

            # ---- phase D: transpose back, normalize, store ----
            with (
                tc.tile_pool(name="tpool", bufs=4, space="PSUM") as tpool,
                tc.tile_pool(name="rpool", bufs=4) as rpool,
            ):
                for h in range(NH):
                    for t in range(QT):
                        tp = tpool.tile([128, 65], F32, tag="tp")
                        nc.tensor.transpose(
                            tp[:], ctxsb[h][:, t * 128:(t + 1) * 128],
                            idsb[0:65, 0:65])
                        rec = rpool.tile([128, 1], F32, tag="rec")
                        nc.vector.reciprocal(rec[:], tp[:, 64:65])
                        nc.vector.tensor_scalar_mul(
                            osb[t][:, h * HD:(h + 1) * HD], tp[:, 0:64], rec[:])
                for t in range(QT):
                    nc.sync.dma_start(out[t * 128:(t + 1) * 128, :], osb[t][:])

    nc.compile()
    return nc


def _get_program(with_mask: bool):
    key = ("prog", with_mask)
    if key not in _cache:
        _cache[key] = _build(with_mask)
    return _cache[key]


def kernel(hidden_states, attention_mask, Wq, bq, Wk, bk, Wv, bv):
    x = np.asarray(hidden_states, np.float32).reshape(S, HID)
    mask = np.asarray(attention_mask, np.float32).reshape(-1)
    if mask.size == 1:
        mask = np.full(S, float(mask[0]), np.float32)
    with_mask = bool(np.any(mask))

    # augmented weights: [3, 1025, 1024] with the bias as the last
    # contraction row; x^T gets a matching ones row.
    w_aug = np.empty((3, KIN, HID), np.float32)
    for i, (W, b) in enumerate(((Wq, bq), (Wk, bk), (Wv, bv))):
        w_aug[i, :HID] = np.asarray(W, np.float32).T
        w_aug[i, HID] = np.asarray(b, np.float32)
    w_aug = w_aug.astype(ml_dtypes.bfloat16)

    ident = np.eye(128, dtype=np.float32)

    nc = _get_program(with_mask)
    in_maps = []
    for c in range(N_CORES):
        xt = np.empty((KIN, SC), np.float32)
        xt[:HID] = x[c * SC:(c + 1) * SC, :].T
        xt[HID] = 1.0
        m = {
            "xt": xt.astype(ml_dtypes.bfloat16),
            "w": w_aug,
            "ident": ident,
        }
        if with_mask:
            m["maskt"] = np.ascontiguousarray(
                mask.reshape(KT, 128).T.astype(np.float32))
        in_maps.append(m)

    res = bass_utils.run_bass_kernel_spmd(nc, in_maps, core_ids=list(range(N_CORES)))
    out = np.concatenate([res.results[c]["out"] for c in range(N_CORES)], axis=0)
    return out.reshape(B, S, HID).astype(np.float32)
